# revision 11
# baseline (speedup 1.0000x reference)
# CondConv2d Trainium2 kernel (Bass/Tile), data-parallel over batch on 8 cores.
#
# Problem (hardcoded):
#   x:               [32, 256, 56, 56] f32
#   routing_weights: [32, 8] f32
#   weight_experts:  [8, 589824] f32      (589824 = 256*256*3*3, co-major)
#   out:             [32, 256, 56, 56] f32
#   out[b] = conv2d(x[b], (routing[b] @ experts).reshape(256,256,3,3), pad=1)
#
# Sharding: 4 samples per core; expert bank replicated.
#
# Host-side prep (layout only, no model compute):
#   - x cast to bf16 and zero-padded to [C, 58, 58]
#   - experts cast to bf16 and permuted to the conv-lhsT layout
#     [E, ci_chunk, co_chunk, ci(128), tap*co(1152)] so the per-tap
#     transposed weight tiles stream straight from DRAM.
#
# Per-core kernel:
#   - DMA T[e][b][a] = [ci, 9*128 co] bf16 supertiles + padded x tiles (HWDGE)
#   - Mix (DVE): per-sample AXPY chains wm[s][b][a] = sum_e r[s,e]*T[e][b][a]
#     (tensor_scalar 4x for e=0, fused scalar_tensor_tensor for e=1..7)
#   - Conv (PE): per (sample, co-chunk, 8-row band): accumulate 9 taps x
#     2 ci-chunks bf16 matmuls (N=448) into PSUM; ACT copy; HWDGE store.

import numpy as np
import ml_dtypes

import concourse.bass as bass
import concourse.mybir as mybir
import concourse.tile as tile
from concourse import bacc, bass_utils

# ---- problem constants ----
B = 32
E = 8
C = 256  # CIN = COUT
H = W = 56
KH = KW = 3
NPARAM = C * C * KH * KW  # 589824
N_CORES = 8
B_LOC = B // N_CORES  # 4 samples per core

PW = W + 2  # 58
PHW = PW * PW  # 3364
NCHUNK = C // 128  # 2 chunks of 128 for both ci and co
ROWS = 8  # output rows per matmul tile
NSP = H // ROWS  # 7 spatial tiles
NTAP = KH * KW  # 9
TCOLS = NTAP * 128  # 1152 cols of a (tap, co) supertile

F32 = mybir.dt.float32
BF16 = mybir.dt.bfloat16
BF16_NP = ml_dtypes.bfloat16


def prep_inputs(x, routing_weights, weight_experts):
    """Host-side layout prep: pad+cast x, permute+cast experts."""
    x = np.asarray(x, dtype=np.float32)
    r = np.ascontiguousarray(np.asarray(routing_weights, dtype=np.float32))
    w = np.asarray(weight_experts, dtype=np.float32)

    x_pad = np.zeros((B, C, PW, PW), dtype=BF16_NP)
    x_pad[:, :, 1 : H + 1, 1 : W + 1] = x

    # [E, (co ci t)] -> [E, b, co, a, ci, t] -> [E, a, b, ci, t, co]
    w6 = w.reshape(E, NCHUNK, 128, NCHUNK, 128, NTAP)
    w_packed = np.ascontiguousarray(
        w6.transpose(0, 3, 1, 4, 5, 2).astype(BF16_NP)
    ).reshape(E, NCHUNK, NCHUNK, 128, TCOLS)  # [E, a, b, ci, (t co)]
    return x_pad, r, w_packed


def build_program(reps=1):
    nc = bacc.Bacc(
        "TRN2",
        target_bir_lowering=False,
        debug=False,
        enable_asserts=False,
    )

    x_d = nc.dram_tensor(
        "x_loc", [B_LOC, C, PW, PW], BF16, kind="ExternalInput"
    ).ap()
    r_d = nc.dram_tensor("r_loc", [B_LOC, E], F32, kind="ExternalInput").ap()
    w_d = nc.dram_tensor(
        "experts", [E, NCHUNK, NCHUNK, 128, TCOLS], BF16, kind="ExternalInput"
    ).ap()
    o_d = nc.dram_tensor("out_loc", [B_LOC, C, H, W], F32, kind="ExternalOutput").ap()

    with tile.TileContext(nc) as tc:
        for _ in range(reps):
            _emit_body(nc, tc, x_d, r_d, w_d, o_d)

    nc.compile()
    return nc


def _emit_body(nc, tc, x_d, r_d, w_d, o_d):
    with (
        tc.tile_pool(name="const", bufs=1) as cpool,
        tc.tile_pool(name="texp", bufs=2 * E * NCHUNK) as tpool,
        tc.tile_pool(name="wmix", bufs=B_LOC * NCHUNK * NCHUNK) as wmpool,
        tc.tile_pool(name="xpad", bufs=B_LOC * NCHUNK) as xppool,
        tc.tile_pool(name="mixtmp", bufs=4) as tmppool,
        tc.tile_pool(name="osb", bufs=4) as opool,
        tc.tile_pool(name="psum_misc", bufs=1, space="PSUM") as pmisc,
        tc.tile_pool(name="psum_c", bufs=7, space="PSUM") as pconv,
    ):
        # ---- prep: broadcast routing weights to all partitions ----
        # r_bc[p, s*8+e] = r[s, e] for every partition p, via K=1 matmul
        # with an all-ones lhsT.
        r_sb = cpool.tile([1, B_LOC * E], F32, tag="r_sb")
        nc.sync.dma_start(out=r_sb, in_=r_d.rearrange("s e -> (s e)")[None, :])
        ones = cpool.tile([1, 128], F32, tag="ones")
        nc.vector.memset(ones, 1.0)
        r_ps = pmisc.tile([128, B_LOC * E], F32, tag="r_ps")
        nc.tensor.matmul(r_ps, lhsT=ones, rhs=r_sb, start=True, stop=True)
        r_bc = cpool.tile([128, B_LOC * E], F32, tag="r_bc")
        nc.scalar.copy(r_bc, r_ps)

        # T[e][b][a]: transposed expert supertiles, straight from DRAM
        tsb = [[[None] * NCHUNK for _ in range(NCHUNK)] for _ in range(E)]
        # wm[b][a][s]: per-sample mixed supertiles
        wm = [[[None] * B_LOC for _ in range(NCHUNK)] for _ in range(NCHUNK)]
        xps = [[None] * NCHUNK for _ in range(B_LOC)]

        def load_T(b):
            for e in range(E):
                for a in range(NCHUNK):
                    ts = tpool.tile(
                        [128, TCOLS], BF16, tag="texp", name=f"t{e}{b}{a}"
                    )
                    nc.sync.dma_start(out=ts, in_=w_d[e, a, b])
                    tsb[e][b][a] = ts

        def load_x(s, head_rows=0):
            """Load a sample's padded chunks; optionally split so the first
            `head_rows` rows land first (unblocks the first conv group)."""
            for a in range(NCHUNK):
                xp = xppool.tile([128, PHW], BF16, tag="xpad", name=f"xp{s}{a}")
                src = x_d[s, a * 128 : (a + 1) * 128].rearrange(
                    "c h w -> c (h w)"
                )
                if head_rows:
                    cut = head_rows * PW
                    nc.sync.dma_start(out=xp[:, :cut], in_=src[:, :cut])
                else:
                    nc.sync.dma_start(out=xp, in_=src)
                xps[s][a] = xp

        def load_x_rest(s, head_rows):
            cut = head_rows * PW
            for a in range(NCHUNK):
                src = x_d[s, a * 128 : (a + 1) * 128].rearrange(
                    "c h w -> c (h w)"
                )
                nc.sync.dma_start(out=xps[s][a][:, cut:], in_=src[:, cut:])

        def mix(s, b):
            # e-outer with a-interleave: both a-chains finish together and
            # each op only needs tsb[e] (pipelines with the T DMA stream).
            for a in range(NCHUNK):
                wm[b][a][s] = wmpool.tile(
                    [128, TCOLS], BF16, tag="wm", name=f"wm{b}{a}{s}"
                )
            for e in range(E):
                sc = r_bc[:, s * E + e : s * E + e + 1]
                for a in range(NCHUNK):
                    wt = wm[b][a][s]
                    if e == 0:
                        nc.vector.tensor_scalar_mul(wt, tsb[0][b][a], sc)
                    else:
                        tmp = tmppool.tile([128, TCOLS], BF16, tag="mixtmp")
                        nc.vector.tensor_scalar_mul(tmp, tsb[e][b][a], sc)
                        nc.vector.tensor_tensor(
                            out=wt, in0=wt, in1=tmp, op=mybir.AluOpType.add
                        )

        # ---- loads + mixing, ordered to unblock conv s0/b0 earliest ----
        HEAD_ROWS = ROWS + 2
        load_x(0, head_rows=HEAD_ROWS)
        load_T(0)
        load_x_rest(0, HEAD_ROWS)
        mix(0, 0)
        load_T(1)
        mix(0, 1)
        load_x(1)
        for s in range(1, B_LOC):
            for b in range(NCHUNK):
                mix(s, b)
            if s + 1 < B_LOC:
                load_x(s + 1)

        # ---- conv phase ----
        for s in range(B_LOC):
            for b in range(NCHUNK):
                for sp in range(NSP):
                    pc = pconv.tile([128, ROWS * W], F32, tag="pconv")
                    i = 0
                    for dy in range(KH):
                        for dx in range(KW):
                            t = dy * KW + dx
                            for a in range(NCHUNK):
                                rhs = xps[s][a].rearrange(
                                    "c (r q) -> c r q", q=PW
                                )[
                                    :,
                                    sp * ROWS + dy : sp * ROWS + dy + ROWS,
                                    dx : dx + W,
                                ]
                                nc.tensor.matmul(
                                    pc,
                                    lhsT=wm[b][a][s][
                                        :, t * 128 : (t + 1) * 128
                                    ],
                                    rhs=rhs,
                                    start=(i == 0),
                                    stop=(i == 2 * NTAP - 1),
                                )
                                i += 1
                    ot = opool.tile([128, ROWS * W], F32, tag="osb")
                    nc.scalar.copy(ot, pc)
                    nc.sync.dma_start(
                        out=o_d[
                            s,
                            b * 128 : (b + 1) * 128,
                            sp * ROWS : (sp + 1) * ROWS,
                            :,
                        ],
                        in_=ot,
                    )


_CACHED_NC = None


def kernel(x, routing_weights, weight_experts, *, trace=False):
    global _CACHED_NC
    x_pad, r, w_packed = prep_inputs(x, routing_weights, weight_experts)

    if _CACHED_NC is None:
        _CACHED_NC = build_program()
    nc = _CACHED_NC

    in_maps = []
    for c in range(N_CORES):
        lo, hi = c * B_LOC, (c + 1) * B_LOC
        in_maps.append(
            {
                "x_loc": x_pad[lo:hi],
                "r_loc": r[lo:hi],
                "experts": w_packed,
            }
        )

    res = bass_utils.run_bass_kernel_spmd(
        nc, in_maps, core_ids=list(range(N_CORES)), trace=trace
    )

    out = np.empty((B, C, H, W), dtype=np.float32)
    for c in range(N_CORES):
        out[c * B_LOC : (c + 1) * B_LOC] = res.results[c]["out_loc"]
    if trace:
        return out, res
    return out
